# revision 18
# baseline (speedup 1.0000x reference)
"""Multi-head attention (b=4, h=8, d=64, n=2048, dim=256) on 8 TRN2 NeuronCores.

Sharding: core c handles batch b=c//2 and head-group g=c%2 (4 heads).
Each core computes its 4 heads' attention plus the partial output
projection y_part = w_out[:, g*256:(g+1)*256] @ attn_out, returned
transposed as yT [n, 256].  Host: y[b] = (yT[2b] + yT[2b+1]).T + b_out.
No cross-core collectives needed.

Per-core pipeline (n=2048, 4 heads processed as 2 row-packed pairs):
  QKV:    q,k via w-stationary matmuls (fp32r), v^T via x-stationary
          matmuls; evict to bf16.  v^T stored with a ones column per head
          ([128, 4*65]) so the AV matmul also computes Z.
  scores: sim_T[j, i] = k^T q  (transposed layout, j on partitions),
          2 heads packed into one [128, 1024] PSUM supertile.
  exp:    single ACT instruction per supertile -> bf16 SBUF.  Softmax
          max-subtraction is skipped (|sim| <~ 8, exp safe in fp32).
  AV:     out_aug^T[d'|Z, i] accumulated over 16 j-tiles, M=65.
  norm:   invZ = 1/Z (DVE), broadcast across partitions (gpsimd),
          multiply (DVE) -> bf16 out_norm in [c_loc, i] layout.
  proj:   yT[i, o] with out_norm stationary, w_outT moving.
"""

import numpy as np
from contextlib import ExitStack

DIM = 256
HEADS = 8
DH = 64
HID = 512
N = 2048
B = 4
SCALE = DH ** -0.5
P = 128
NI = N // 512   # 4 i-chunks of 512
NJ = N // P     # 16 j-tiles of 128

_CACHE = {}


def _build_nc(repeat=1):
    if repeat in _CACHE:
        return _CACHE[repeat]
    import concourse.tile as tile
    from concourse import bacc, mybir

    f32 = mybir.dt.float32
    bf16 = mybir.dt.bfloat16
    Exp = mybir.ActivationFunctionType.Exp

    nc = bacc.Bacc("TRN2", target_bir_lowering=False, debug=False)
    x_d = nc.dram_tensor("x", [DIM, N], f32, kind="ExternalInput").ap()
    wqk_d = nc.dram_tensor("wqkT", [DIM, 512], f32, kind="ExternalInput").ap()
    wv_d = nc.dram_tensor("wvT", [DIM, 256], f32, kind="ExternalInput").ap()
    wo_d = nc.dram_tensor("woutT", [DIM, 256], f32, kind="ExternalInput").ap()
    y_d = nc.dram_tensor("yT", [N, DIM], f32, kind="ExternalOutput").ap()

    with tile.TileContext(nc) as tc, ExitStack() as ctx:
        persist = ctx.enter_context(tc.tile_pool(name="persist", bufs=1))

        # small weights first (they gate the first QKV matmuls)
        wqk = []
        for r in range(2):
            wf = persist.tile([P, 512], f32, tag=f"wqkf{r}", name=f"wqkf{r}")
            nc.sync.dma_start(wf[:], wqk_d[r * P:(r + 1) * P, :])
            wt = persist.tile([P, 512], bf16, tag=f"wqk{r}", name=f"wqk{r}")
            nc.vector.tensor_copy(wt[:], wf[:])
            wqk.append(wt)
        # x: chunk 0 arrives as two small DMAs (gates the first matmuls); the
        # rest as two big DMAs.  The HWDGE ring serializes DMA dispatch at
        # ~625ns/instruction, so fewer+bigger is better past the head.
        xsb = {}
        for r in range(2):
            xf = persist.tile([P, 512], f32, tag=f"xf{r}_0", name=f"xf{r}_0")
            nc.sync.dma_start(xf[:], x_d[r * P:(r + 1) * P, 0:512])
            xt = persist.tile([P, 512], bf16, tag=f"x{r}_0", name=f"xs{r}_0")
            nc.vector.tensor_copy(xt[:], xf[:])
            xsb[(r, 0)] = xt
        for r in range(2):
            xf = persist.tile([P, 1536], f32, tag=f"xfr{r}", name=f"xfr{r}")
            nc.sync.dma_start(xf[:], x_d[r * P:(r + 1) * P, 512:N])
            xt = persist.tile([P, 1536], bf16, tag=f"xr{r}", name=f"xsr{r}")
            nc.gpsimd.tensor_copy(xt[:], xf[:])
            for c in range(1, NI):
                xsb[(r, c)] = xt[:, (c - 1) * 512:c * 512]
        wv = []
        for r in range(2):
            wf = persist.tile([P, 256], f32, tag=f"wvf{r}", name=f"wvf{r}")
            nc.sync.dma_start(wf[:], wv_d[r * P:(r + 1) * P, :])
            wt = persist.tile([P, 256], bf16, tag=f"wv{r}", name=f"wv{r}")
            nc.vector.tensor_copy(wt[:], wf[:])
            wv.append(wt)
        wob = []
        for r in range(2):
            wf = persist.tile([P, 256], f32, tag=f"wo{r}", name=f"wof{r}")
            nc.scalar.dma_start(wf[:], wo_d[r * P:(r + 1) * P, :])
            wb = persist.tile([P, 256], bf16, tag=f"wob{r}", name=f"wob{r}")
            nc.vector.tensor_copy(wb[:], wf[:])
            wob.append(wb)

        # ---- Stage A: minimal upfront QKV; the rest is interleaved into the
        # first attention loop so the PE's in-order stream reaches the first
        # scores matmul (and hence the first ACT exp) as early as possible.
        # qkt[(m, c)]: m=0 q heads01, m=1 q heads23, m=2 k heads01, m=3 k heads23
        qkt = {}
        vT = [None] * NJ

        def emit_qk(pool, m, c):
            ps = pool.tile([P, 512], f32, tag=pool._qkv_tag, name="qkps")
            for r in range(2):
                nc.tensor.matmul(
                    ps[:],
                    wqk[r][:, m * P:(m + 1) * P],
                    xsb[(r, c)],
                    start=(r == 0), stop=(r == 1),
                )
            t = persist.tile([P, 512], bf16, tag=f"qk{m}_{c}", name=f"qk{m}_{c}")
            nc.vector.tensor_copy(t[:], ps[:])
            qkt[(m, c)] = t

        def emit_v(pool, j):
            ps = pool.tile([P, 256], f32, tag=pool._qkv_tag, name="vps")
            for r in range(2):
                nc.tensor.matmul(
                    ps[:],
                    xsb[(r, j // 4)][:, (j % 4) * P:(j % 4 + 1) * P],
                    wv[r][:],
                    start=(r == 0), stop=(r == 1),
                )
            t = persist.tile([P, 4 * 65], bf16, tag=f"vT{j}", name=f"vT{j}")
            tv = t[:].rearrange("p (h w) -> p h w", h=4)
            nc.gpsimd.memset(tv[:, :, 64:65], 1.0)
            nc.vector.tensor_copy(tv[:, :, 0:64], ps[:].rearrange("p (h w) -> p h w", h=4))
            vT[j] = t

        with tc.tile_pool(name="qkvps", bufs=2, space="PSUM") as qp:
            qp._qkv_tag = "qkps"
            emit_qk(qp, 2, 0)   # k heads 0,1 chunk 0
            emit_qk(qp, 0, 0)   # q heads 0,1 chunk 0

        # ---- Stage B: attention + projection ----
        on = []
        for p2 in range(2):
            t = persist.tile([P, N], bf16, tag=f"on{p2}", name=f"on{p2}")
            on.append(t)
        small = ctx.enter_context(tc.tile_pool(name="small", bufs=2))
        expool = ctx.enter_context(tc.tile_pool(name="expool", bufs=3))
        simp = ctx.enter_context(tc.tile_pool(name="simp", bufs=2, space="PSUM"))
        avp = ctx.enter_context(tc.tile_pool(name="avp", bufs=3, space="PSUM"))
        yp = ctx.enter_context(tc.tile_pool(name="yp", bufs=1, space="PSUM"))
        yout = ctx.enter_context(tc.tile_pool(name="yout", bufs=2))

        yp._qkv_tag = "ypsum"
        # Deferred QKV pieces, interleaved into the early attention loops.
        # Emitted between exp(jt) and av(jt), where the PE is waiting on ACT
        # anyway.  Constraints: v(j) at or before iteration j; k01_cX before
        # jt=4X; k-tiles of a pair before that pair's loop; q_cX before ic=X.
        deferred = {
            (0, 0): {
                0: [("v", 0)],
                1: [("v", 1), ("v", 2)],
                2: [("v", 3), ("qk", 2, 1)],
                3: [("v", 4), ("v", 5)],
                4: [("v", 6), ("qk", 2, 2)],
                5: [("v", 7), ("v", 8)],
                6: [("v", 9), ("qk", 2, 3)],
                7: [("v", 10), ("v", 11)],
                8: [("v", 12), ("qk", 3, 0)],
                9: [("v", 13), ("v", 14)],
                10: [("v", 15), ("qk", 3, 1)],
                11: [("qk", 3, 2)],
                12: [("qk", 3, 3)],
                13: [("qk", 1, 0)],
                14: [("qk", 0, 1)],
            },
            (0, 1): {
                0: [("qk", 1, 1)],
                2: [("qk", 0, 2)],
                4: [("qk", 1, 2)],
            },
            (1, 0): {
                0: [("qk", 0, 3)],
                2: [("qk", 1, 3)],
            },
        }

        for rep in range(repeat):
            for ic in range(NI):
                for p in range(2):
                    qt = qkt[(p, ic)]
                    av_a = avp.tile([65, 512], f32, tag="av", name="av_a")
                    av_b = avp.tile([65, 512], f32, tag="av", name="av_b")
                    dmap = deferred.get((ic, p), {}) if rep == 0 else {}
                    for jt in range(NJ):
                        sim = simp.tile([P, 1024], f32, tag="sim", name="sim")
                        kt = qkt[(2 + p, jt // 4)]
                        ko = (jt % 4) * P
                        nc.tensor.matmul(sim[:, 0:512], kt[0:64, ko:ko + P],
                                         qt[0:64, :], start=True, stop=True)
                        nc.tensor.matmul(sim[:, 512:1024], kt[64:128, ko:ko + P],
                                         qt[64:128, :], start=True, stop=True)
                        ex = expool.tile([P, 1024], bf16, tag="ex", name="ex")
                        nc.scalar.activation(ex[:], sim[:], Exp)
                        for piece in dmap.get(jt, []):
                            if piece[0] == "v":
                                emit_v(yp, piece[1])
                            else:
                                emit_qk(yp, piece[1], piece[2])
                        nc.tensor.matmul(av_a[:], vT[jt][:, (2 * p) * 65:(2 * p) * 65 + 65],
                                         ex[:, 0:512], start=(jt == 0), stop=(jt == NJ - 1))
                        nc.tensor.matmul(av_b[:], vT[jt][:, (2 * p + 1) * 65:(2 * p + 1) * 65 + 65],
                                         ex[:, 512:1024], start=(jt == 0), stop=(jt == NJ - 1))
                    for h2, av in ((0, av_a), (1, av_b)):
                        iz = small.tile([1, 512], f32, tag="iz", name="iz")
                        nc.vector.reciprocal(iz[:], av[64:65, :])
                        bc = small.tile([64, 512], f32, tag="bc", name="bc")
                        nc.gpsimd.partition_broadcast(bc[:], iz[:])
                        nc.vector.tensor_mul(
                            on[p][h2 * 64:(h2 + 1) * 64, ic * 512:(ic + 1) * 512],
                            av[0:64, :], bc[:])
                for s in range(4):
                    i0 = ic * 512 + s * P
                    pyp = yp.tile([P, 256], f32, tag="ypsum", name="ypsum")
                    for ct in range(2):
                        nc.tensor.matmul(pyp[:], on[ct][:, i0:i0 + P], wob[ct][:],
                                         start=(ct == 0), stop=(ct == 1))
                    ysb = yout.tile([P, 256], f32, tag="ysb", name="ysb")
                    nc.vector.tensor_copy(ysb[:], pyp[:])
                    nc.sync.dma_start(y_d[i0:i0 + P, :], ysb[:])

    nc.compile()
    _CACHE[repeat] = nc
    return nc


def _shard_inputs(x, w_qkv, w_out):
    in_maps = []
    for c in range(8):
        b, g = c // 2, c % 2
        wq = w_qkv[g * 256:(g + 1) * 256] * SCALE
        wk = w_qkv[512 + g * 256:512 + (g + 1) * 256]
        wvv = w_qkv[1024 + g * 256:1024 + (g + 1) * 256]
        in_maps.append({
            "x": np.ascontiguousarray(x[b], dtype=np.float32),
            "wqkT": np.ascontiguousarray(np.concatenate([wq, wk], 0).T, dtype=np.float32),
            "wvT": np.ascontiguousarray(wvv.T, dtype=np.float32),
            "woutT": np.ascontiguousarray(w_out[:, g * 256:(g + 1) * 256].T, dtype=np.float32),
        })
    return in_maps


def kernel(x, w_qkv, w_out, b_out):
    from concourse.bass_utils import run_bass_kernel_spmd
    x = np.asarray(x, dtype=np.float32)
    w_qkv = np.asarray(w_qkv, dtype=np.float32)
    w_out = np.asarray(w_out, dtype=np.float32)
    b_out = np.asarray(b_out, dtype=np.float32)

    nc = _build_nc()
    in_maps = _shard_inputs(x, w_qkv, w_out)
    res = run_bass_kernel_spmd(nc, in_maps, core_ids=list(range(8)))
    y = np.empty((B, DIM, N), np.float32)
    for b in range(B):
        yT = res.results[2 * b]["yT"] + res.results[2 * b + 1]["yT"]
        y[b] = yT.T + b_out[:, None]
    return y
